# revision 28
# baseline (speedup 1.0000x reference)
"""Grouped linear (MoE expert GEMM) on 8 NeuronCores, expert-parallel.

Problem: hidden_states [16384, 2048] f32, weight [8, 2048, 2048] f32,
tokens_per_expert [8] = 2048 each (balanced). Output [16384, 2048] f32 with
out[g*2048+t, o] = sum_i x[g*2048+t, i] * weight[g, o, i].

Sharding: expert-parallel — core g gets expert g's weight [2048, 2048] and its
2048 routed tokens; each core runs one 2048x2048x2048 GEMM. No collectives.

Per-core kernel: mixed precision. The bulk of the contraction runs in bf16
(rounded on the host), which streams the PE at the 216 ns/matmul floor
(512 cols @ 2.4 GHz + NX issue). The trailing K=512 of every accumulation
group except the four startup-critical ones runs as two fp8-e4m3 DoubleRow
matmuls instead — each covers K=256 at ~220 ns (true 2x; the second weight
load hides under the first's stream). The fp8 operands carry a power-of-two
scale split (x/8, w*8) so their product accumulates into the same PSUM group
as the bf16 matmuls at scale 1. Measured rel err 1.58e-2 vs the 2e-2 gate,
deterministic (hardware fp8/bf16 numerics match the ml_dtypes emulation).
X^T and W^T are fully resident in SBUF. W slices stream on the sync HWDGE
queue, X tiles and outputs on the scalar queue, ordered so the first groups'
data lands first (startup is bounded by the ~0.65 us/DMA trigger-issue rate
and the ~390 GB/s per-core HBM read bandwidth, both measured on hardware).
"""

import numpy as np
import ml_dtypes

G = 8
TPG = 2048  # tokens per expert (= per core)
IN = 2048
OUT = 2048
P = 128
KM = IN // P  # 16 contraction chunks of 128
TT = TPG // P  # 16 token tiles of 128
ON = 4  # number of output-column chunks
OW = OUT // ON  # 512
K8 = 512  # trailing K range computed in fp8 (DoubleRow) on non-critical groups
K0 = IN - K8  # 1536
NC8 = K8 // 256  # DoubleRow matmuls per group (each covers K=256)
S8 = 8.0  # fp8 scale split: x/S8 and w*S8, net product scale 1

_nc_cache = {}


def _build_nc():
    import concourse.bacc as bacc
    import concourse.mybir as mybir
    import concourse.tile as tile

    if "nc" in _nc_cache:
        return _nc_cache["nc"]

    f32 = mybir.dt.float32
    bf16 = mybir.dt.bfloat16
    f8 = mybir.dt.float8e4

    nc = bacc.Bacc(None, target_bir_lowering=False)

    # xt[p, tt, km, tl] = x_core[tt*128+tl, km*128+p]   (X^T, k on partitions)
    xt = nc.dram_tensor("xt", [P, TT, KM, P], bf16, kind="ExternalInput")
    # wt[p, km, oi, o] = w_core[oi*512+o, km*128+p]     (W^T, k on partitions)
    wt = nc.dram_tensor("wt", [P, KM, ON, OW], bf16, kind="ExternalInput")
    # fp8 copies of the trailing K8 columns (k = K0 + ko*128 + ki), used by
    # one DoubleRow matmul per group on output chunks 1..3:
    # xt8[ki, tt, c, ko, tl] = x[tt*128+tl, K0+c*256+ko*128+ki] / S8
    xt8 = nc.dram_tensor("xt8", [P, TT, NC8, 2, P], f8, kind="ExternalInput")
    # wt8[ki, oi, c, ko, o] = w[oi*512+o, K0+c*256+ko*128+ki] * S8
    wt8 = nc.dram_tensor("wt8", [P, ON, NC8, 2, OW], f8, kind="ExternalInput")
    # out[tt, p, o] = C[tt*128+p, o]
    out = nc.dram_tensor("out", [TT, P, OUT], f32, kind="ExternalOutput")

    with tile.TileContext(nc) as tc:
        with (
            tc.tile_pool(name="xpool", bufs=1) as xpool,
            tc.tile_pool(name="wpool", bufs=1) as wpool,
            tc.tile_pool(name="opool", bufs=8) as opool,
            tc.tile_pool(name="ppool", bufs=8, space="PSUM") as ppool,
            tc.tile_pool(name="f8pool", bufs=1) as f8pool,
        ):
            xtiles = [
                xpool.tile([P, KM, P], bf16, name=f"x_sb{tt}", tag=f"x{tt}")
                for tt in range(TT)
            ]
            # bf16 W slices; for output chunks 1..3 the last K8 columns are
            # covered by the fp8 path instead, so those slices don't exist.
            KM8 = KM - K8 // P  # 14
            wtiles = [
                [
                    (
                        wpool.tile(
                            [P, OW], bf16, name=f"w_sb{km}_{oi}", tag=f"w{km}_{oi}"
                        )
                        if (oi == 0 or km < KM8)
                        else None
                    )
                    for oi in range(ON)
                ]
                for km in range(KM)
            ]

            # Critical path first: x tile 0 on the scalar HWDGE queue, the 16
            # W k-slices of output chunk 0 (consumed in km order by the first
            # accumulation group) on the sync HWDGE queue. Bulk follows.
            x8_sb = f8pool.tile([P, TT, NC8, 2, P], f8, name="x8_sb", tag="x8")
            w8_sb = [
                f8pool.tile([P, NC8, 2, OW], f8, name=f"w8_sb{oi}", tag=f"w8{oi}")
                for oi in range(ON)
            ]

            nc.scalar.dma_start(out=xtiles[0][:], in_=xt[:, 0])
            for km in range(KM):
                nc.sync.dma_start(out=wtiles[km][0][:], in_=wt[:, km, 0])
            for tt in range(1, TT):
                nc.scalar.dma_start(out=xtiles[tt][:], in_=xt[:, tt])
            for km in range(KM8):
                nc.sync.dma_start(out=wtiles[km][1][:], in_=wt[:, km, 1])
            nc.sync.dma_start(out=x8_sb[:], in_=xt8[:])
            for oi in range(ON):
                nc.sync.dma_start(out=w8_sb[oi][:], in_=wt8[:, oi])
            for oi in range(2, ON):
                for km in range(KM8):
                    nc.sync.dma_start(out=wtiles[km][oi][:], in_=wt[:, km, oi])

            for oi in range(ON):
                for tt in range(TT):
                    # groups on the startup critical path (output chunk 0,
                    # first 4 token tiles) stay pure bf16 so they never wait
                    # on the fp8 tensors; everything else uses the fp8 tail.
                    use_f8 = oi > 0 or tt >= 4
                    kmax = KM8 if use_f8 else KM
                    psum = ppool.tile([P, OW], f32, name="psum", tag="psum")
                    for km in range(kmax):
                        nc.tensor.matmul(
                            out=psum[:],
                            lhsT=xtiles[tt][:, km, :],
                            rhs=wtiles[km][oi][:],
                            start=(km == 0),
                            stop=(km == KM - 1),
                        )
                    if use_f8:
                        # trailing K8 contraction in fp8 DoubleRow MMs
                        # (each covers K=256 at ~one bf16 matmul's cost)
                        for c in range(NC8):
                            nc.tensor.matmul(
                                out=psum[:],
                                lhsT=x8_sb[:, tt, c],
                                rhs=w8_sb[oi][:, c],
                                start=False,
                                stop=(c == NC8 - 1),
                                perf_mode=mybir.MatmulPerfMode.DoubleRow,
                            )
                    o_sb = opool.tile([P, OW], f32, name="o_sb", tag="o_sb")
                    nc.vector.tensor_copy(out=o_sb[:], in_=psum[:])
                    nc.scalar.dma_start(
                        out=out[tt, :, oi * OW : (oi + 1) * OW], in_=o_sb[:]
                    )

    nc.compile()
    _nc_cache["nc"] = nc
    return nc


def _shard_inputs(hidden_states, weight):
    """Host-side reshuffle + bf16/fp8 rounding into the kernel's DRAM layouts."""
    bf16 = ml_dtypes.bfloat16
    f8 = ml_dtypes.float8_e4m3
    xf = np.asarray(hidden_states, dtype=np.float32)
    wf = np.asarray(weight, dtype=np.float32)
    x = xf.astype(bf16)
    w = wf.astype(bf16)
    in_maps = []
    for g in range(G):
        xg = x[g * TPG : (g + 1) * TPG]  # [2048, 2048]
        # [tt, tl, km, p] -> [p, tt, km, tl]
        xtg = np.ascontiguousarray(xg.reshape(TT, P, KM, P).transpose(3, 0, 2, 1))
        wg = w[g]  # [out, in]
        # [oi, o, km, p] -> [p, km, oi, o]
        wtg = np.ascontiguousarray(wg.reshape(ON, OW, KM, P).transpose(3, 2, 0, 1))
        # fp8 copies of the trailing K8 columns, quantized from full fp32
        x8 = (xf[g * TPG : (g + 1) * TPG, K0:] / S8).astype(f8)  # [2048, K8]
        # [tt, tl, c, ko, ki] -> [ki, tt, c, ko, tl]
        NC8 = K8 // 256
        xt8g = np.ascontiguousarray(
            x8.reshape(TT, P, NC8, 2, P).transpose(4, 0, 2, 3, 1)
        )
        w8 = (wf[g][:, K0:] * S8).astype(f8)  # [2048, K8]
        # [oi, o, c, ko, ki] -> [ki, oi, c, ko, o]
        wt8g = np.ascontiguousarray(
            w8.reshape(ON, OW, NC8, 2, P).transpose(4, 0, 2, 3, 1)
        )
        in_maps.append({"xt": xtg, "wt": wtg, "xt8": xt8g, "wt8": wt8g})
    return in_maps


def _run(hidden_states, weight, trace=False, tmpdir=None):
    from concourse.bass_utils import run_bass_kernel_spmd

    nc = _build_nc()
    in_maps = _shard_inputs(hidden_states, weight)
    res = run_bass_kernel_spmd(
        nc, in_maps, core_ids=list(range(G)), trace=trace, tmpdir=tmpdir
    )
    outs = [
        np.asarray(res.results[g]["out"]).reshape(TPG, OUT) for g in range(G)
    ]
    full = np.concatenate(outs, axis=0)
    return full, res


def kernel(hidden_states, weight, tokens_per_expert=None, **_ignored):
    out, _ = _run(hidden_states, weight, trace=False)
    return out


# revision 29
# speedup vs baseline: 1.0418x; 1.0418x over previous
"""Grouped linear (MoE expert GEMM) on 8 NeuronCores, expert-parallel.

Problem: hidden_states [16384, 2048] f32, weight [8, 2048, 2048] f32,
tokens_per_expert [8] = 2048 each (balanced). Output [16384, 2048] f32 with
out[g*2048+t, o] = sum_i x[g*2048+t, i] * weight[g, o, i].

Sharding: expert-parallel — core g gets expert g's weight [2048, 2048] and its
2048 routed tokens; each core runs one 2048x2048x2048 GEMM. No collectives.

Per-core kernel: mixed precision. The bulk of the contraction runs in bf16
(rounded on the host), which streams the PE at the 216 ns/matmul floor
(512 cols @ 2.4 GHz + NX issue). The trailing K=512 of every accumulation
group except the four startup-critical ones runs as two fp8-e4m3 DoubleRow
matmuls instead — each covers K=256 at ~220 ns (true 2x; the second weight
load hides under the first's stream). The fp8 operands carry a power-of-two
scale split (x/8, w*8) so their product accumulates into the same PSUM group
as the bf16 matmuls at scale 1. Measured rel err 1.58e-2 vs the 2e-2 gate,
deterministic (hardware fp8/bf16 numerics match the ml_dtypes emulation).
X^T and W^T are fully resident in SBUF. W slices stream on the sync HWDGE
queue, X tiles and outputs on the scalar queue, ordered so the first groups'
data lands first (startup is bounded by the ~0.65 us/DMA trigger-issue rate
and the ~390 GB/s per-core HBM read bandwidth, both measured on hardware).
"""

import numpy as np
import ml_dtypes

G = 8
TPG = 2048  # tokens per expert (= per core)
IN = 2048
OUT = 2048
P = 128
KM = IN // P  # 16 contraction chunks of 128
TT = TPG // P  # 16 token tiles of 128
ON = 4  # number of output-column chunks
OW = OUT // ON  # 512
K8 = 512  # trailing K range computed in fp8 (DoubleRow) on non-critical groups
K0 = IN - K8  # 1536
NC8 = K8 // 256  # DoubleRow matmuls per group (each covers K=256)
S8 = 8.0  # fp8 scale split: x/S8 and w*S8, net product scale 1

_nc_cache = {}


def _build_nc():
    import concourse.bacc as bacc
    import concourse.mybir as mybir
    import concourse.tile as tile

    if "nc" in _nc_cache:
        return _nc_cache["nc"]

    f32 = mybir.dt.float32
    bf16 = mybir.dt.bfloat16
    f8 = mybir.dt.float8e4

    nc = bacc.Bacc(None, target_bir_lowering=False)

    # xt[p, tt, km, tl] = x_core[tt*128+tl, km*128+p]   (X^T, k on partitions)
    xt = nc.dram_tensor("xt", [P, TT, KM, P], bf16, kind="ExternalInput")
    # wt[p, km, oi, o] = w_core[oi*512+o, km*128+p]     (W^T, k on partitions)
    wt = nc.dram_tensor("wt", [P, KM, ON, OW], bf16, kind="ExternalInput")
    # fp8 copies of the trailing K8 columns (k = K0 + ko*128 + ki), used by
    # one DoubleRow matmul per group on output chunks 1..3:
    # xt8[ki, tt, c, ko, tl] = x[tt*128+tl, K0+c*256+ko*128+ki] / S8
    xt8 = nc.dram_tensor("xt8", [P, TT, NC8, 2, P], f8, kind="ExternalInput")
    # wt8[ki, oi, c, ko, o] = w[oi*512+o, K0+c*256+ko*128+ki] * S8
    wt8 = nc.dram_tensor("wt8", [P, ON, NC8, 2, OW], f8, kind="ExternalInput")
    # out[tt, p, o] = C[tt*128+p, o]
    out = nc.dram_tensor("out", [TT, P, OUT], f32, kind="ExternalOutput")

    with tile.TileContext(nc) as tc:
        with (
            tc.tile_pool(name="xpool", bufs=1) as xpool,
            tc.tile_pool(name="wpool", bufs=1) as wpool,
            tc.tile_pool(name="opool", bufs=8) as opool,
            tc.tile_pool(name="ppool", bufs=8, space="PSUM") as ppool,
            tc.tile_pool(name="f8pool", bufs=1) as f8pool,
        ):
            xtiles = [
                xpool.tile([P, KM, P], bf16, name=f"x_sb{tt}", tag=f"x{tt}")
                for tt in range(TT)
            ]
            # bf16 W slices; for output chunks 1..3 the last K8 columns are
            # covered by the fp8 path instead, so those slices don't exist.
            KM8 = KM - K8 // P  # 14
            wtiles = [
                [
                    (
                        wpool.tile(
                            [P, OW], bf16, name=f"w_sb{km}_{oi}", tag=f"w{km}_{oi}"
                        )
                        if (oi == 0 or km < KM8)
                        else None
                    )
                    for oi in range(ON)
                ]
                for km in range(KM)
            ]

            # Critical path first: x tile 0 on the scalar HWDGE queue, the 16
            # W k-slices of output chunk 0 (consumed in km order by the first
            # accumulation group) on the sync HWDGE queue. Bulk follows.
            x8_sb = f8pool.tile([P, TT, NC8, 2, P], f8, name="x8_sb", tag="x8")
            w8_sb = [
                f8pool.tile([P, NC8, 2, OW], f8, name=f"w8_sb{oi}", tag=f"w8{oi}")
                for oi in range(ON)
            ]

            nc.scalar.dma_start(out=xtiles[0][:], in_=xt[:, 0])
            for km in range(KM):
                nc.sync.dma_start(out=wtiles[km][0][:], in_=wt[:, km, 0])
            for tt in range(1, TT):
                nc.scalar.dma_start(out=xtiles[tt][:], in_=xt[:, tt])
            nc.sync.dma_start(out=x8_sb[:], in_=xt8[:])
            nc.sync.dma_start(out=w8_sb[0][:], in_=wt8[:, 0])
            for km in range(KM8):
                nc.sync.dma_start(out=wtiles[km][1][:], in_=wt[:, km, 1])
            for oi in range(1, ON):
                nc.sync.dma_start(out=w8_sb[oi][:], in_=wt8[:, oi])
            for oi in range(2, ON):
                for km in range(KM8):
                    nc.sync.dma_start(out=wtiles[km][oi][:], in_=wt[:, km, oi])

            for oi in range(ON):
                for tt in range(TT):
                    # groups on the startup critical path (output chunk 0,
                    # first 4 token tiles) stay pure bf16 so they never wait
                    # on the fp8 tensors; everything else uses the fp8 tail.
                    use_f8 = oi > 0 or tt >= 4
                    kmax = KM8 if use_f8 else KM
                    psum = ppool.tile([P, OW], f32, name="psum", tag="psum")
                    for km in range(kmax):
                        nc.tensor.matmul(
                            out=psum[:],
                            lhsT=xtiles[tt][:, km, :],
                            rhs=wtiles[km][oi][:],
                            start=(km == 0),
                            stop=(km == KM - 1),
                        )
                    if use_f8:
                        # trailing K8 contraction in fp8 DoubleRow MMs
                        # (each covers K=256 at ~one bf16 matmul's cost)
                        for c in range(NC8):
                            nc.tensor.matmul(
                                out=psum[:],
                                lhsT=x8_sb[:, tt, c],
                                rhs=w8_sb[oi][:, c],
                                start=False,
                                stop=(c == NC8 - 1),
                                perf_mode=mybir.MatmulPerfMode.DoubleRow,
                            )
                    o_sb = opool.tile([P, OW], f32, name="o_sb", tag="o_sb")
                    nc.vector.tensor_copy(out=o_sb[:], in_=psum[:])
                    nc.scalar.dma_start(
                        out=out[tt, :, oi * OW : (oi + 1) * OW], in_=o_sb[:]
                    )

    nc.compile()
    _nc_cache["nc"] = nc
    return nc


def _shard_inputs(hidden_states, weight):
    """Host-side reshuffle + bf16/fp8 rounding into the kernel's DRAM layouts."""
    bf16 = ml_dtypes.bfloat16
    f8 = ml_dtypes.float8_e4m3
    xf = np.asarray(hidden_states, dtype=np.float32)
    wf = np.asarray(weight, dtype=np.float32)
    x = xf.astype(bf16)
    w = wf.astype(bf16)
    in_maps = []
    for g in range(G):
        xg = x[g * TPG : (g + 1) * TPG]  # [2048, 2048]
        # [tt, tl, km, p] -> [p, tt, km, tl]
        xtg = np.ascontiguousarray(xg.reshape(TT, P, KM, P).transpose(3, 0, 2, 1))
        wg = w[g]  # [out, in]
        # [oi, o, km, p] -> [p, km, oi, o]
        wtg = np.ascontiguousarray(wg.reshape(ON, OW, KM, P).transpose(3, 2, 0, 1))
        # fp8 copies of the trailing K8 columns, quantized from full fp32
        x8 = (xf[g * TPG : (g + 1) * TPG, K0:] / S8).astype(f8)  # [2048, K8]
        # [tt, tl, c, ko, ki] -> [ki, tt, c, ko, tl]
        NC8 = K8 // 256
        xt8g = np.ascontiguousarray(
            x8.reshape(TT, P, NC8, 2, P).transpose(4, 0, 2, 3, 1)
        )
        w8 = (wf[g][:, K0:] * S8).astype(f8)  # [2048, K8]
        # [oi, o, c, ko, ki] -> [ki, oi, c, ko, o]
        wt8g = np.ascontiguousarray(
            w8.reshape(ON, OW, NC8, 2, P).transpose(4, 0, 2, 3, 1)
        )
        in_maps.append({"xt": xtg, "wt": wtg, "xt8": xt8g, "wt8": wt8g})
    return in_maps


def _run(hidden_states, weight, trace=False, tmpdir=None):
    from concourse.bass_utils import run_bass_kernel_spmd

    nc = _build_nc()
    in_maps = _shard_inputs(hidden_states, weight)
    res = run_bass_kernel_spmd(
        nc, in_maps, core_ids=list(range(G)), trace=trace, tmpdir=tmpdir
    )
    outs = [
        np.asarray(res.results[g]["out"]).reshape(TPG, OUT) for g in range(G)
    ]
    full = np.concatenate(outs, axis=0)
    return full, res


def kernel(hidden_states, weight, tokens_per_expert=None, **_ignored):
    out, _ = _run(hidden_states, weight, trace=False)
    return out
